# revision 39
# baseline (speedup 1.0000x reference)
"""AxialSelfAttention2d Trainium2 kernel (v2).

Strategy (8 NeuronCores, SPMD, two launches):
  - Stage 1 (row attention, attend along L): shard over S -> 16 rows/core.
  - Host reshard, apply gamma1/beta1 exactly.
  - Stage 2 (col attention, attend along S): shard over L -> 32 cols/core,
    per-core tensor laid out [D, l, s] so both stages run the same kernel
    parameterized by (R rows, Lr row-length).

Device kernel per stage (per core, N = R*Lr = 4096 positions):
  inputs : x_bf [D, N] bf16 channel-major,
           xT [N, D] bf16 position-major WITH the v-bias pre-added
           (attn_out = num'/den + b_v exactly, since softmax weights sum
           to 1 -- so b_v joins the residual),
           wqkT [D, 256] bf16 (q,k 1x1-conv weights),
           wvT [D, 128] bf16 (v weights, per-head-contiguous, no bias:
           b_k drops entirely -- softmax over j is invariant to the
           q_i.b_k + b_q.b_k terms -- and b_v moved to the residual),
           bq [D, 1] f32 (q bias; b_q.k_j varies over j so it stays).
  output : out [N, D] bf16 = LayerNorm(x + rowattn(x)) without gamma/beta.

  Pipeline per unit (2 position blocks): scores (PE, 2-PSUM-bank tiles)
  -> exp: ACT Exp for most tiles, DVE Schraudolph (i16 = s*A + B, bits
  reinterpreted as bf16; the constant factor cancels in softmax) for the
  rest to balance the two engines -> AV matmul into a separate 1-bank
  PSUM tile with an interleaved ones column per head giving the softmax
  denominator -> reciprocal+multiply (DVE) -> residual add (DVE, bf16 2x)
  -> bn_stats/aggr -> LN scale (GPSIMD) -> bf16 DMA out.

  A single activation-table load (Exp/Ln/Identity all steered into
  natural_log_exp_and_others) is hoisted to t=0 by a warmup exp.
"""

import os

os.environ.setdefault("MYCRO_LOCAL_CACHE", "1")

import numpy as np
import ml_dtypes

import concourse.bass as bass
import concourse.bacc as bacc
import concourse.tile as tile
from concourse import mybir

H, CH, D = 4, 32, 128
S, L = 128, 256
EPS = 1e-5
NCORES = 8
F32, BF16, I16 = mybir.dt.float32, mybir.dt.bfloat16, mybir.dt.int16
AF = mybir.ActivationFunctionType
OP = mybir.AluOpType

TRACE = False
# Schraudolph bf16 exp: i16 = rint(s * SCH_A + SCH_B), bits viewed as bf16.
SCH_A = 128.0 / float(np.log(2.0))
SCH_B = 16250.5
EX_I16 = True  # ex tiles typed i16 + bitcast views (needed for the DVE path)
WARMUP = True  # t=0 dummy exp to hoist the act-table load
AV_PAD = False  # pad av PSUM tiles to a full 2KB bank
DEN_MM = False  # denominator via per-head ones-matmul instead of strided v evict
Y2_F32 = False  # y2 tile in f32 (bn_stats/gpsimd read f32 as in v1)
ALL_F32 = False  # v1 dtype profile: xT/out dram f32, y/y2/xn f32
STOP_AFTER = "full"  # debug: qkv | score | exp | av | div | full
PS_BUFS = 3  # ps tag depth (2-bank slots)
TPOS_ZERO = False  # force tile_position=(0,0) on score matmuls
EXP_DVE_MOD = 3  # score tiles with t % MOD == MOD-1 take the DVE exp path
K_EVICT = "vector"  # k eviction engine (no bias needed)
V_EVICT = "scalar"  # v eviction engine
RES_ENGINE = "vector"  # residual add engine
XN_ENGINE = "gpsimd"  # LN scale engine
FIN = 8  # finalize/store granularity (blocks)
MID_U = 6  # unit index at which the second half QKV is emitted
DIV_PRIO = 60
EXP_PRIO = 60

_nc_cache = {}


def _stage_body(tc, d, R, Lr):
    nc = tc.nc
    N = R * Lr
    NB = N // 128  # 32 position blocks
    JB = Lr // 128  # key blocks per attention row (2 stage1, 1 stage2)
    NT = N * 4 // 1024  # score tiles (32 stage1: (row, jb); 16 stage2: 2 rows)
    NU = 16  # attention units; unit u owns position blocks 2u, 2u+1

    import contextlib

    with contextlib.ExitStack() as ctx:
        cpool = ctx.enter_context(tc.tile_pool(name="consts", bufs=1))
        big = ctx.enter_context(tc.tile_pool(name="big", bufs=1))
        sm = ctx.enter_context(tc.tile_pool(name="small", bufs=1))
        pp = ctx.enter_context(tc.tile_pool(name="ps", bufs=1, space="PSUM"))

        # ---- warmup: host the single act-table load at t=0 ----
        if WARMUP:
            wu = cpool.tile([128, 1], F32)
            nc.vector.memset(wu[:], 0.0)
            wu2 = cpool.tile([128, 1], F32)
            nc.scalar.activation(wu2[:], wu[:], AF.Exp)

        # ---- constants (wqk first: first matmul needs it + x chunk 0) ----
        wqk_sb = cpool.tile([128, 256], BF16)
        nc.sync.dma_start(out=wqk_sb[:], in_=d["wqkT"][:])
        wv_sb = cpool.tile([128, 128], BF16)
        bq_sb = cpool.tile([128, 1], F32)
        eps_sb = cpool.tile([128, 1], F32)
        nc.vector.memset(eps_sb[:], EPS)

        # ---- inputs ----
        x_sb = big.tile([128, N], BF16)
        XC = N // 4
        nc.sync.dma_start(out=x_sb[:, 0:XC], in_=d["x_bf"][:, 0:XC])
        nc.sync.dma_start(out=wv_sb[:], in_=d["wvT"][:])
        nc.sync.dma_start(out=bq_sb[:], in_=d["bq"][:])
        for q in range(1, 4):
            nc.sync.dma_start(
                out=x_sb[:, q * XC : (q + 1) * XC],
                in_=d["x_bf"][:, q * XC : (q + 1) * XC],
            )
        xT_sb = big.tile([128, NB, 128], F32 if ALL_F32 else BF16)
        xT_v = d["xT"].rearrange("(nb p) d -> p nb d", p=128)
        for q in range(4):
            nc.sync.dma_start(
                out=xT_sb[:, q * 8 : (q + 1) * 8, :], in_=xT_v[:, q * 8 : (q + 1) * 8, :]
            )

        # ---- persistent intermediates ----
        qk_sb = big.tile([128, 2, N], BF16)
        # head-major copies: head h's 32 channels at partitions 0..32, so
        # score matmuls run on PE quadrant (0,0) -- per-head quadrants would
        # need one PSUM bank per head (concurrent quadrant streams must hit
        # distinct banks), which the 2-bank score tiles can't provide.
        q2_sb = big.tile([128, 4, N], BF16)
        k2_sb = big.tile([128, 4, N], BF16)
        if DEN_MM:
            v_sb = big.tile([128, NB, 128], BF16)
            ones_sb = cpool.tile([128, 1], BF16)
            nc.vector.memset(ones_sb[:], 1.0)
        else:
            v_sb = big.tile([128, NB, 4, 33], BF16)
            # denominator ones columns (evictions fill [:, :, :, 0:32])
            nc.vector.memset(v_sb[:, :, :, 32:33], 1.0)
        y_sb = big.tile([128, NB, 128], F32 if ALL_F32 else BF16)
        y2_sb = big.tile([128, NB, 128], F32 if (Y2_F32 or ALL_F32) else BF16)
        xn_sb = big.tile([128, NB, 128], F32 if ALL_F32 else BF16)
        mv_sb = sm.tile([128, NB, 2], F32)
        rstd_sb = sm.tile([128, NB], F32)

        def emit_qk_chunk(cc):
            # 1024 columns of q then k; q gets its bias on ACT, k is a pure
            # copy (b_k cancels in the softmax) on K_EVICT.
            for ob in range(2):
                ps = pp.tile([128, 2, 512], F32, tag="ps", bufs=PS_BUFS, name=f"qk{ob}{cc}")
                for b in range(2):
                    nc.tensor.matmul(
                        ps[:, b, :],
                        lhsT=wqk_sb[:, ob * 128 : (ob + 1) * 128],
                        rhs=x_sb[:, (cc * 2 + b) * 512 : (cc * 2 + b + 1) * 512],
                        start=True,
                        stop=True,
                    )
                outv = qk_sb[:, ob, cc * 1024 : (cc + 1) * 1024].rearrange(
                    "p (a b) -> p a b", b=512
                )
                if ob == 0:
                    nc.scalar.activation(outv, ps[:], AF.Identity, bias=bq_sb[:])
                elif K_EVICT == "scalar":
                    nc.scalar.activation(outv, ps[:], AF.Identity)
                else:
                    getattr(nc, K_EVICT).tensor_copy(outv, ps[:])
            # head-major remap of this chunk (SBUF->SBUF DMA partition move)
            sl = slice(cc * 1024, (cc + 1) * 1024)
            for h in range(4):
                nc.sync.dma_start(
                    out=q2_sb[0:32, h, sl], in_=qk_sb[32 * h : 32 * h + 32, 0, sl]
                )
                nc.sync.dma_start(
                    out=k2_sb[0:32, h, sl], in_=qk_sb[32 * h : 32 * h + 32, 1, sl]
                )

        def emit_v(t):
            # 8 position blocks -> v_sb[:, 8t:8t+8, :, 0:32] (strided out).
            ps = pp.tile([128, 8, 128], F32, tag="ps", bufs=PS_BUFS, name=f"v{t}")
            for u in range(8):
                nb = t * 8 + u
                nc.tensor.matmul(
                    ps[:, u, :],
                    lhsT=x_sb[:, nb * 128 : (nb + 1) * 128],
                    rhs=wv_sb[:],
                    start=True,
                    stop=True,
                )
            if DEN_MM:
                inv = ps[:]
                outv = v_sb[:, t * 8 : (t + 1) * 8, :]
            else:
                inv = ps.rearrange("p u (h c) -> p u h c", c=32)
                outv = v_sb[:, t * 8 : (t + 1) * 8, :, 0:32]
            if V_EVICT == "scalar":
                nc.scalar.activation(outv, inv, AF.Identity)
            else:
                getattr(nc, V_EVICT).tensor_copy(outv, inv)

        def emit_score_tile(t):
            """Score tile t: [128 keys, 4 heads, 256 queries] = 2 PSUM banks.
            Stage1: t = (row r, key-block jb) -> 4 matmuls of 256 queries.
            Stage2: t = 2 rows -> 8 matmuls of 128 queries.
            Returns the exp'd bf16 (or bit-tricked i16) SBUF tile."""
            sc = pp.tile([128, 4, 256], F32, tag="ps", bufs=PS_BUFS, name=f"sc{t}")
            if JB == 2:
                r, jb = t // 2, t % 2
                for h in range(4):
                    nc.tensor.matmul(
                        sc[:, h, :],
                        lhsT=k2_sb[
                            0:32, h, r * Lr + jb * 128 : r * Lr + (jb + 1) * 128
                        ],
                        rhs=q2_sb[0:32, h, r * Lr : (r + 1) * Lr],
                        start=True,
                        stop=True,
                    )
            else:
                scv = sc.rearrange("p h (r q) -> p h r q", q=128)
                for rr in range(2):
                    r = 2 * t + rr
                    for h in range(4):
                        nc.tensor.matmul(
                            scv[:, h, rr, :],
                            lhsT=k2_sb[0:32, h, r * Lr : (r + 1) * Lr],
                            rhs=q2_sb[0:32, h, r * Lr : (r + 1) * Lr],
                            start=True,
                            stop=True,
                        )
            ex = sm.tile([128, 4, 256], I16 if EX_I16 else BF16, tag="ex", bufs=4,
                         name=f"ex{t}")
            if STOP_AFTER == "score":
                return ex
            on_dve = (t % EXP_DVE_MOD) == (EXP_DVE_MOD - 1)
            import contextlib as _ctx
            with tc.high_priority(offset=EXP_PRIO) if EXP_PRIO else _ctx.nullcontext():
                if on_dve:
                    nc.vector.tensor_scalar(
                        ex[:], sc[:], SCH_A, SCH_B, OP.mult, OP.add
                    )
                else:
                    out_ap = ex[:].bitcast(BF16) if EX_I16 else ex[:]
                    nc.scalar.activation(out_ap, sc[:], AF.Exp)
            return ex

        def emit_unit(u):
            """Attention unit u: position blocks 2u, 2u+1 (one stage1 row of
            256 queries, or two stage2 rows of 128)."""
            if AV_PAD:
                av_t = pp.tile([128, 2, 256], F32, tag="av", bufs=2, name=f"av{u}")
                av = av_t[:, :, 0:132]
            else:
                av_t = pp.tile([128, 2, 132], F32, tag="av", bufs=2, name=f"av{u}")
                av = av_t[:]

            def av_mm(ib, h, lhsT, vb, start, stop):
                if DEN_MM:
                    nc.tensor.matmul(
                        av[:, ib, 33 * h : 33 * h + 32],
                        lhsT=lhsT,
                        rhs=v_sb[:, vb, 32 * h : 32 * h + 32],
                        start=start,
                        stop=stop,
                    )
                    nc.tensor.matmul(
                        av[:, ib, 33 * h + 32 : 33 * h + 33],
                        lhsT=lhsT,
                        rhs=ones_sb[:],
                        start=start,
                        stop=stop,
                    )
                else:
                    nc.tensor.matmul(
                        av[:, ib, 33 * h : 33 * h + 33],
                        lhsT=lhsT,
                        rhs=v_sb[:, vb, h, :],
                        start=start,
                        stop=stop,
                    )

            if JB == 2:
                exs = [emit_score_tile(2 * u), emit_score_tile(2 * u + 1)]
                if STOP_AFTER in ("exp", "score"):
                    return
                for ib in range(2):
                    for h in range(4):
                        for jb in range(2):
                            lhsT = (
                                exs[jb][:].bitcast(BF16) if EX_I16 else exs[jb][:]
                            )[:, h, ib * 128 : (ib + 1) * 128]
                            av_mm(ib, h, lhsT, 2 * u + jb, jb == 0, jb == 1)
            else:
                ex = emit_score_tile(u)
                if STOP_AFTER in ("exp", "score"):
                    return
                exv = (ex[:].bitcast(BF16) if EX_I16 else ex[:]).rearrange(
                    "p h (r q) -> p h r q", q=128
                )
                for rr in range(2):
                    for h in range(4):
                        av_mm(rr, h, exv[:, h, rr, :], 2 * u + rr, True, True)
            if STOP_AFTER == "av":
                return
            # softmax divide: y = num * (1/den); high priority to release the
            # av PSUM slot quickly.
            av4 = av.rearrange("p b (h c) -> p b h c", c=33)
            rt = sm.tile([128, 2, 4], F32, tag="rt", bufs=4, name=f"rt{u}")
            yv = y_sb[:, 2 * u : 2 * u + 2, :].rearrange("p b (h c) -> p b h c", c=32)
            with tc.high_priority(offset=DIV_PRIO):
                nc.vector.reciprocal(rt[:], av4[:, :, :, 32])
                nc.vector.tensor_tensor(
                    yv,
                    av4[:, :, :, 0:32],
                    rt[:, :, :, None].to_broadcast([128, 2, 4, 32]),
                    OP.mult,
                )
            if STOP_AFTER == "div":
                return
            # residual add: all-bf16 SBUF -> DVE 2x mode
            getattr(nc, RES_ENGINE).tensor_tensor(
                y2_sb[:, 2 * u : 2 * u + 2, :],
                y_sb[:, 2 * u : 2 * u + 2, :],
                xT_sb[:, 2 * u : 2 * u + 2, :],
                OP.add,
            )
            for nb in (2 * u, 2 * u + 1):
                st = sm.tile([128, 6], F32, tag="st", bufs=6, name=f"st{nb}")
                nc.vector.bn_stats(st[:], y2_sb[:, nb, :])
                nc.vector.bn_aggr(mv_sb[:, nb, :], st[:])

            done = 2 * (u + 1)
            if done % FIN == 0:
                gg = done // FIN - 1
                sl = slice(gg * FIN, gg * FIN + FIN)
                # rstd = exp(-0.5*ln(var+eps)); Ln/Exp share the one act table
                lnv = sm.tile([128, FIN], F32, tag="std", bufs=2, name=f"lnv{gg}")
                nc.scalar.activation(lnv[:], mv_sb[:, sl, 1], AF.Ln, bias=eps_sb[:])
                nc.scalar.activation(rstd_sb[:, sl], lnv[:], AF.Exp, scale=-0.5)
                xn_eng = "vector" if done == NB else XN_ENGINE
                for nb in range(gg * FIN, gg * FIN + FIN):
                    getattr(nc, xn_eng).tensor_scalar(
                        xn_sb[:, nb, :],
                        y2_sb[:, nb, :],
                        mv_sb[:, nb, 0:1],
                        rstd_sb[:, nb : nb + 1],
                        OP.subtract,
                        OP.mult,
                    )
                out_v = d["out"].rearrange("(nb p) d -> p nb d", p=128)
                nc.sync.dma_start(out=out_v[:, sl, :], in_=xn_sb[:, sl, :])

        # ---- driver ----
        emit_qk_chunk(0)
        emit_qk_chunk(1)
        emit_v(0)
        emit_v(1)
        if STOP_AFTER != "qkv":
            for u in range(NU):
                if u == MID_U:
                    emit_qk_chunk(2)
                    emit_qk_chunk(3)
                    emit_v(2)
                    emit_v(3)
                emit_unit(u)


def _build_stage(R, Lr):
    N = R * Lr
    nc = bacc.Bacc("TRN2", target_bir_lowering=False, debug=False)
    IO_DT = F32 if ALL_F32 else BF16
    d = {
        "x_bf": nc.dram_tensor("x_bf", [D, N], BF16, kind="ExternalInput").ap(),
        "xT": nc.dram_tensor("xT", [N, D], IO_DT, kind="ExternalInput").ap(),
        "wqkT": nc.dram_tensor("wqkT", [D, 2 * D], BF16, kind="ExternalInput").ap(),
        "wvT": nc.dram_tensor("wvT", [D, D], BF16, kind="ExternalInput").ap(),
        "bq": nc.dram_tensor("bq", [D, 1], F32, kind="ExternalInput").ap(),
        "out": nc.dram_tensor("out", [N, D], IO_DT, kind="ExternalOutput").ap(),
    }
    with tile.TileContext(nc) as tc:
        _stage_body(tc, d, R, Lr)
    _compile_with_shared_act_table(nc)
    return nc


def _compile_with_shared_act_table(nc):
    """Steer the act-table-load pass to the one set containing Exp, Ln AND
    Identity (natural_log_exp_and_others), so the kernel does exactly one
    table load (~1.3us). Identity is in every set, so it must be masked out
    of the others or the first Identity picks a different set."""
    import concourse.hw_specs as hws

    orig = hws.get_activation_tables
    orig_bacc = bacc.get_activation_tables
    tabs = dict(orig(nc.m.arch))
    want = {AF.Exp, AF.Ln}
    mask = {AF.Exp, AF.Ln, AF.Identity}
    shared = [n for n, fs in tabs.items() if want <= fs]
    if shared:
        keep = shared[0]
        masked = {n: (fs if n == keep else (fs - mask)) for n, fs in tabs.items()}
        patched = lambda arch, _m=masked: _m
        hws.get_activation_tables = patched
        bacc.get_activation_tables = patched
    try:
        nc.compile()
    finally:
        hws.get_activation_tables = orig
        bacc.get_activation_tables = orig_bacc


def _get_stage(R, Lr):
    key = (R, Lr)
    if key not in _nc_cache:
        _nc_cache[key] = _build_stage(R, Lr)
    return _nc_cache[key]


def _prep_weights(w, b):
    """Host-side packing of the [384, 128] qkv conv weights.

    Returns wqkT [D, 256] bf16 (q, k columns), wvT [D, 128] bf16
    (head-contiguous v weights, no ones, no bias), bq [D, 1] f32, and
    bv [D] f32 (to be folded into the residual input xT)."""
    w = np.asarray(w, np.float32)
    b = np.asarray(b, np.float32)
    wqkT = w[0 : 2 * D].T.astype(ml_dtypes.bfloat16)  # [D, 256]
    wvT = w[2 * D : 3 * D].T.astype(ml_dtypes.bfloat16)  # [D, 128]
    bq = np.ascontiguousarray(b[0:D, None])  # [D, 1]
    bv = b[2 * D : 3 * D]  # [D]
    return wqkT, wvT, bq, bv


class _PjrtStage:
    """Cached sharded PJRT executable for one Bass program (8-core SPMD)."""

    def __init__(self, nc):
        import jax
        from jax.sharding import Mesh, PartitionSpec
        from jax.experimental.shard_map import shard_map
        from concourse import bass2jax, mybir as _mybir

        bass2jax.install_neuronx_cc_hook()
        self.nc = nc
        part_name = nc.partition_id_tensor.name if nc.partition_id_tensor else None
        in_names, out_names, out_avals = [], [], []
        for alloc in nc.m.functions[0].allocations:
            if not isinstance(alloc, _mybir.MemoryLocationSet):
                continue
            name = alloc.memorylocations[0].name
            if alloc.kind == "ExternalInput":
                if name != part_name:
                    in_names.append(name)
            elif alloc.kind == "ExternalOutput":
                out_names.append(name)
                out_avals.append(
                    jax.core.ShapedArray(
                        tuple(alloc.tensor_shape), _mybir.dt.np(alloc.dtype)
                    )
                )
        self.in_names, self.out_names, self.out_avals = in_names, out_names, out_avals
        n_params = len(in_names)
        all_names = list(in_names + out_names)
        if part_name is not None:
            all_names.append(part_name)
        all_names = tuple(all_names)

        def _body(*args):
            operands = list(args)
            if part_name is not None:
                operands.append(bass2jax.partition_id_tensor())
            return tuple(
                bass2jax._bass_exec_p.bind(
                    *operands,
                    out_avals=tuple(out_avals),
                    in_names=all_names,
                    out_names=tuple(out_names),
                    lowering_input_output_aliases=(),
                    sim_require_finite=True,
                    sim_require_nnan=True,
                    nc=nc,
                )
            )

        devices = jax.devices()[:NCORES]
        mesh = Mesh(np.asarray(devices), ("core",))
        nio = n_params + len(out_names)
        self._fn = jax.jit(
            shard_map(
                _body,
                mesh=mesh,
                in_specs=(PartitionSpec("core"),) * nio,
                out_specs=(PartitionSpec("core"),) * len(out_names),
                check_rep=False,
            ),
            donate_argnums=tuple(range(n_params, nio)),
            keep_unused=True,
        )

    def concat_inputs(self, in_maps):
        return [
            np.concatenate([np.asarray(m[name]) for m in in_maps], axis=0)
            for name in self.in_names
        ]

    def run(self, concat_in):
        zeros = [
            np.zeros((NCORES * a.shape[0], *a.shape[1:]), a.dtype)
            for a in self.out_avals
        ]
        out = self._fn(*concat_in, *zeros)
        return [o for o in out]

    def __call__(self, in_maps):
        out = self.run(self.concat_inputs(in_maps))
        a = self.out_avals[0]
        return np.asarray(out[0]).reshape(NCORES, *a.shape)


_stage_runners = {}


def _get_runner(R, Lr):
    key = (R, Lr)
    if key not in _stage_runners:
        _stage_runners[key] = _PjrtStage(_get_stage(R, Lr))
    return _stage_runners[key]


def _run_stage(R, Lr, shards_cm, wqkT, wvT, bq, bv):
    """shards_cm: list of 8 channel-major [D, N] f32 arrays. Returns
    [8, N, D] f32 (upcast from the device's bf16)."""
    xt_dt = np.float32 if ALL_F32 else ml_dtypes.bfloat16
    in_maps = []
    for xs in shards_cm:
        xTb = xs.T + bv[None, :]  # fold v bias into the residual input
        in_maps.append(
            {
                "x_bf": xs.astype(ml_dtypes.bfloat16),
                "xT": np.ascontiguousarray(xTb).astype(xt_dt),
                "wqkT": wqkT,
                "wvT": wvT,
                "bq": bq,
            }
        )
    return _get_runner(R, Lr)(in_maps).astype(np.float32)


def kernel(**inputs):
    x = np.asarray(inputs["x"], np.float32)  # [1, D, S, L]
    g1 = np.asarray(inputs["gamma1"], np.float32)
    b1 = np.asarray(inputs["beta1"], np.float32)
    g2 = np.asarray(inputs["gamma2"], np.float32)
    b2 = np.asarray(inputs["beta2"], np.float32)

    # ---- stage 1: row attention, shard over S ----
    wqkT, wvT, bq, bv = _prep_weights(inputs["w_row"], inputs["b_row"])
    Rs = S // NCORES
    shards = [
        np.ascontiguousarray(x[0][:, c * Rs : (c + 1) * Rs, :]).reshape(D, Rs * L)
        for c in range(NCORES)
    ]
    xn1 = _run_stage(Rs, L, shards, wqkT, wvT, bq, bv)  # [8, Rs*L, D]
    out1 = xn1.reshape(S, L, D) * g1[None, None, :] + b1[None, None, :]

    # ---- stage 2: col attention, shard over L, per-core layout [D, l, s] ----
    wqkT, wvT, bq, bv = _prep_weights(inputs["w_col"], inputs["b_col"])
    Rl = L // NCORES
    shards = [
        np.ascontiguousarray(
            out1[:, c * Rl : (c + 1) * Rl, :].transpose(2, 1, 0)
        ).reshape(D, Rl * S)
        for c in range(NCORES)
    ]
    xn2 = _run_stage(Rl, S, shards, wqkT, wvT, bq, bv)  # [8, Rl*S, D]
    full = np.concatenate(
        [xn2[c].reshape(Rl, S, D) for c in range(NCORES)], axis=0
    )  # [L, S, D]
    out = full.transpose(1, 0, 2) * g2[None, None, :] + b2[None, None, :]  # [S, L, D]
    return np.ascontiguousarray(out.transpose(2, 0, 1))[None].astype(np.float32)


# revision 53
# speedup vs baseline: 1.0876x; 1.0876x over previous
"""AxialSelfAttention2d Trainium2 kernel (v4 = v1 skeleton + surgical wins).

Strategy (8 NeuronCores, SPMD, two launches):
  - Stage 1 (row attention, attend along L): shard over S -> 16 rows/core.
  - Host reshard, apply gamma1/beta1 exactly.
  - Stage 2 (col attention, attend along S): shard over L -> 32 cols/core,
    per-core layout [D, l, s] so both stages run the same kernel
    parameterized by (R rows, Lr row-length).

Device kernel per stage (per core, N = R*Lr = 4096 positions):
  inputs : x_bf [D, N] bf16 channel-major, xT [N, D] bf16 position-major
           with the v-bias pre-added (softmax weights sum to 1, so b_v
           joins the residual exactly), wqkT [D, 256] bf16, wvT [D, 128]
           bf16 (no bias, no ones), bqk [D, 2] f32.
  output : out [N, D] bf16 = LayerNorm(x + attn(x)) without gamma/beta.

  vs the original baseline:
  - single activation-table load at t=0 (Exp/Ln/Identity all masked into
    natural_log_exp_and_others + a dependency-free warmup exp).
  - a tunable fraction of the per-group exps runs on DVE via the
    Schraudolph bit-trick (i16 = s*A + B viewed as bf16; the constant
    factor cancels in softmax), balancing ACT vs DVE.
  - v eviction is a strided pure copy on ACT (b_v folded into xT on the
    host; denominator ones columns memset once), freeing DVE time.
  - y/y2/xn/xT are bf16: the residual add runs in DVE 2x mode and the
    xT-in/out DMAs halve.
"""

import os

os.environ.setdefault("MYCRO_LOCAL_CACHE", "1")

import numpy as np
import ml_dtypes

import concourse.bass as bass
import concourse.bacc as bacc
import concourse.tile as tile
from concourse import mybir

H, CH, D = 4, 32, 128
S, L = 128, 256
EPS = 1e-5
NCORES = 8
F32, BF16, I16 = mybir.dt.float32, mybir.dt.bfloat16, mybir.dt.int16
AF = mybir.ActivationFunctionType
OP = mybir.AluOpType

TRACE = False
SCH_A = 128.0 / float(np.log(2.0))
SCH_B = 16250.5
# groups whose exp runs on DVE (Schraudolph), per stage: {Lr: set of g}
SCH_GROUPS = {256: set(), 128: set()}
RES_ENGINE = "vector"  # residual add engine: vector | gpsimd
XN_ENGINE = {256: "gpsimd", 128: "gpsimd"}  # LN scale engine per stage
V_EVICT = {256: "vector", 128: "vector"}  # v eviction engine per stage
FIN_BLOCKS = 4  # finalize/store granularity (blocks)
K_EVICT_ACT = True  # stage2: k eviction on ACT (vs DVE)
EXP_BUFS = 4
X_CHUNKS = 4
DIV_PRIO = 60
EXP_PRIO = 60

_nc_cache = {}


def _stage_body(tc, d, R, Lr):
    nc = tc.nc
    N = R * Lr
    JB = Lr // 128          # j-blocks per attention row (2 for Lr=256, 1 for 128)
    NB = N // 128           # 32 position blocks
    RPT = 512 // (Lr * JB)  # attention rows per score tile (1 / 4)
    M = RPT * JB            # score subunits per bank == y-blocks per group
    G = R // RPT            # number of score groups
    sch_groups = SCH_GROUPS[Lr]

    import contextlib

    with contextlib.ExitStack() as ctx:
        cpool = ctx.enter_context(tc.tile_pool(name="consts", bufs=1))
        big = ctx.enter_context(tc.tile_pool(name="big", bufs=1))
        sm = ctx.enter_context(tc.tile_pool(name="small", bufs=1))
        pp = ctx.enter_context(tc.tile_pool(name="ps", bufs=1, space="PSUM"))

        # ---- warmup: hoist the single act-table load to t=0 ----
        wu = cpool.tile([128, 1], F32)
        nc.vector.memset(wu[:], 0.0)
        wu2 = cpool.tile([128, 1], F32)
        nc.scalar.activation(wu2[:], wu[:], AF.Exp)

        # ---- constants (wqk first: the first matmul needs it + x chunk 0) ----
        wqk_sb = cpool.tile([128, 256], BF16)
        nc.sync.dma_start(out=wqk_sb[:], in_=d["wqkT"][:])
        wv_sb = cpool.tile([128, 128], BF16)
        bqk_sb = cpool.tile([128, 2], F32)
        eps_sb = cpool.tile([128, 1], F32)
        nc.vector.memset(eps_sb[:], EPS)

        # ---- inputs ----
        x_sb = big.tile([128, N], BF16)
        XC = N // X_CHUNKS
        nc.sync.dma_start(out=x_sb[:, 0:XC], in_=d["x_bf"][:, 0:XC])
        nc.sync.dma_start(out=wv_sb[:], in_=d["wvT"][:])
        nc.sync.dma_start(out=bqk_sb[:], in_=d["bqk"][:])
        for q in range(1, X_CHUNKS):
            nc.sync.dma_start(
                out=x_sb[:, q * XC : (q + 1) * XC],
                in_=d["x_bf"][:, q * XC : (q + 1) * XC],
            )
        xT_sb = big.tile([128, NB, 128], BF16)
        xT_v = d["xT"].rearrange("(nb p) d -> p nb d", p=128)

        def emit_xT(q):  # deferred: residual input isn't needed until late
            nc.sync.dma_start(
                out=xT_sb[:, q * 8 : (q + 1) * 8, :], in_=xT_v[:, q * 8 : (q + 1) * 8, :]
            )

        # ---- persistent intermediates ----
        qk_sb = big.tile([128, 2, N], BF16)
        v_sb = big.tile([128, NB, 4, 33], BF16)
        # denominator ones columns, written once (evictions fill [.., 0:32])
        nc.vector.memset(v_sb[:, :, :, 32:33], 1.0)
        y_sb = big.tile([128, NB, 128], BF16)
        y2_sb = big.tile([128, NB, 128], BF16)
        xn_sb = big.tile([128, NB, 128], BF16)
        mv_sb = sm.tile([128, NB, 2], F32)
        rstd_sb = sm.tile([128, NB], F32)

        def emit_qk(cc):
            for ob in range(2):
                mmps = pp.tile([128, 4, 512], F32, tag="ps", bufs=2, name=f"qkps{ob}{cc}")
                for b in range(4):
                    nc.tensor.matmul(
                        mmps[:, b, :],
                        lhsT=wqk_sb[:, ob * 128 : (ob + 1) * 128],
                        rhs=x_sb[:, (cc * 4 + b) * 512 : (cc * 4 + b + 1) * 512],
                        start=True,
                        stop=True,
                    )
                outv = qk_sb[:, ob, cc * 2048 : (cc + 1) * 2048].rearrange(
                    "p (a b) -> p a b", b=512
                )
                on_act = (ob == 0) or (K_EVICT_ACT and JB == 1)
                if on_act:
                    nc.scalar.activation(
                        outv, mmps[:], AF.Identity, bias=bqk_sb[:, ob : ob + 1]
                    )
                else:
                    nc.vector.tensor_scalar(
                        outv, mmps[:], bqk_sb[:, ob : ob + 1], 0.0, OP.add, OP.add
                    )

        def emit_v(t):
            vps = pp.tile([128, 4, 512], F32, tag="ps", bufs=2, name=f"vps{t}")
            vv = vps.rearrange("p b (s x) -> p (b s) x", x=256)
            for u in range(8):
                nb = t * 8 + u
                nc.tensor.matmul(
                    vv[:, u, 0:128],
                    lhsT=x_sb[:, nb * 128 : (nb + 1) * 128],
                    rhs=wv_sb[:],
                    start=True,
                    stop=True,
                )
            inv = vv[:, :, 0:128].rearrange("p u (h c) -> p u h c", c=32)
            outv = v_sb[:, t * 8 : (t + 1) * 8, :, 0:32]
            if V_EVICT[Lr] == "scalar":
                nc.scalar.activation(outv, inv, AF.Identity)
            else:
                nc.vector.tensor_copy(outv, inv)

        # ---- attention + LN ----
        def emit_attention(g):
            rows = [g * RPT + rp for rp in range(RPT)]
            sc = pp.tile([128, 4, M, Lr], F32, tag="ps", bufs=2, name=f"sc{g}")
            sc_flat = sc.rearrange("p h m i -> p h (m i)")
            for rp, r in enumerate(rows):
                for jb in range(JB):
                    kk = rp * JB + jb
                    for h in range(4):
                        nc.tensor.matmul(
                            sc[:, h, kk, :],
                            lhsT=qk_sb[
                                32 * h : 32 * h + 32,
                                1,
                                r * Lr + jb * 128 : r * Lr + (jb + 1) * 128,
                            ],
                            rhs=qk_sb[32 * h : 32 * h + 32, 0, r * Lr : (r + 1) * Lr],
                            start=True,
                            stop=True,
                            tile_position=(32 * h, 0),
                        )
            ex_i = sm.tile([128, 4, M, Lr], I16, tag="exp", bufs=EXP_BUFS, name=f"ex{g}")
            ex = ex_i[:].bitcast(BF16)
            with tc.high_priority(offset=EXP_PRIO) if EXP_PRIO else contextlib.nullcontext():
                if g in sch_groups:
                    nc.vector.tensor_scalar(
                        ex_i[:], sc[:], SCH_A, SCH_B, OP.mult, OP.add
                    )
                else:
                    nc.scalar.activation(ex, sc[:], AF.Exp)

            # AV: unit u -> y block nb = g*M + u; av psum reuses sc bank u.
            for u in range(M):
                r = rows[u // JB]
                ib = u % JB
                for h in range(4):
                    for jb in range(JB):
                        kk = (u // JB) * JB + jb
                        nc.tensor.matmul(
                            sc_flat[:, u, 33 * h : 33 * h + 33],
                            lhsT=ex[:, h, kk, ib * 128 : (ib + 1) * 128],
                            rhs=v_sb[:, r * JB + jb, h, :],
                            start=(jb == 0),
                            stop=(jb == JB - 1),
                        )
            # divide: y = num * (1/denom), batched over all M units.
            av4 = sc_flat[:, 0:M, 0:132].rearrange("p u (h c) -> p u h c", c=33)
            rt = sm.tile([128, M, 4], F32, tag="rt", bufs=4, name=f"rt{g}")
            yv = y_sb[:, g * M : (g + 1) * M, :].rearrange("p u (h c) -> p u h c", c=32)
            with tc.high_priority(offset=DIV_PRIO):
                nc.vector.reciprocal(rt[:], av4[:, :, :, 32])
                nc.vector.tensor_tensor(
                    yv, av4[:, :, :, 0:32],
                    rt[:, :, :, None].to_broadcast([128, M, 4, 32]), OP.mult,
                )
            # residual add: all-bf16 SBUF -> DVE 2x mode
            getattr(nc, RES_ENGINE).tensor_tensor(
                y2_sb[:, g * M : (g + 1) * M, :],
                y_sb[:, g * M : (g + 1) * M, :],
                xT_sb[:, g * M : (g + 1) * M, :],
                OP.add,
            )
            # LN stats per block
            for u in range(M):
                nb = g * M + u
                st = sm.tile([128, 6], F32, tag="st", bufs=6, name=f"st{g}_{u}")
                nc.vector.bn_stats(st[:], y2_sb[:, nb, :])
                nc.vector.bn_aggr(mv_sb[:, nb, :], st[:])

            # finalize + store every FIN blocks
            FIN = FIN_BLOCKS
            done = (g + 1) * M
            if done % FIN == 0:
                gg = done // FIN - 1
                sl = slice(gg * FIN, gg * FIN + FIN)
                # rstd = exp(-0.5*ln(var+eps)): Ln and Exp share one table set
                lnv = sm.tile([128, FIN], F32, tag="std", bufs=2, name=f"lnv{gg}")
                nc.scalar.activation(lnv[:], mv_sb[:, sl, 1], AF.Ln, bias=eps_sb[:])
                nc.scalar.activation(rstd_sb[:, sl], lnv[:], AF.Exp, scale=-0.5)
                xn_eng = "vector" if done == NB else XN_ENGINE[Lr]  # tail on DVE
                for nb in range(gg * FIN, gg * FIN + FIN):
                    getattr(nc, xn_eng).tensor_scalar(
                        xn_sb[:, nb, :],
                        y2_sb[:, nb, :],
                        mv_sb[:, nb, 0:1],
                        rstd_sb[:, nb : nb + 1],
                        OP.subtract,
                        OP.mult,
                    )
                out_v = d["out"].rearrange("(nb p) d -> p nb d", p=128)
                nc.sync.dma_start(out=out_v[:, sl, :], in_=xn_sb[:, sl, :])

        # driver: interleave per half so PSUM slots pipeline across phases
        emit_qk(0)
        emit_v(0)
        emit_v(1)
        for q in range(4):
            emit_xT(q)
        for half in range(2):
            for g in range(half * (G // 2), (half + 1) * (G // 2)):
                emit_attention(g)
            if half == 0:
                emit_qk(1)
                emit_v(2)
                emit_v(3)


def _build_stage(R, Lr):
    N = R * Lr
    nc = bacc.Bacc("TRN2", target_bir_lowering=False, debug=False)
    d = {
        "x_bf": nc.dram_tensor("x_bf", [D, N], BF16, kind="ExternalInput").ap(),
        "xT": nc.dram_tensor("xT", [N, D], BF16, kind="ExternalInput").ap(),
        "wqkT": nc.dram_tensor("wqkT", [D, 2 * D], BF16, kind="ExternalInput").ap(),
        "wvT": nc.dram_tensor("wvT", [D, D], BF16, kind="ExternalInput").ap(),
        "bqk": nc.dram_tensor("bqk", [D, 2], F32, kind="ExternalInput").ap(),
        "out": nc.dram_tensor("out", [N, D], BF16, kind="ExternalOutput").ap(),
    }
    with tile.TileContext(nc) as tc:
        _stage_body(tc, d, R, Lr)
    _compile_with_shared_act_table(nc)
    return nc


def _compile_with_shared_act_table(nc):
    """Steer the act-table-load pass to the one set containing Exp, Ln AND
    Identity (natural_log_exp_and_others) so exactly one table load runs."""
    import concourse.hw_specs as hws

    orig = hws.get_activation_tables
    orig_bacc = bacc.get_activation_tables
    tabs = dict(orig(nc.m.arch))
    want = {AF.Exp, AF.Ln}
    mask = {AF.Exp, AF.Ln, AF.Identity}
    shared = [n for n, fs in tabs.items() if want <= fs]
    if shared:
        keep = shared[0]
        masked = {n: (fs if n == keep else (fs - mask)) for n, fs in tabs.items()}
        patched = lambda arch, _m=masked: _m
        hws.get_activation_tables = patched
        bacc.get_activation_tables = patched
    try:
        nc.compile()
    finally:
        hws.get_activation_tables = orig
        bacc.get_activation_tables = orig_bacc


def _get_stage(R, Lr):
    key = (R, Lr)
    if key not in _nc_cache:
        _nc_cache[key] = _build_stage(R, Lr)
    return _nc_cache[key]


def _prep_weights(w, b):
    """Host-side packing of the [384, 128] qkv conv weights."""
    w = np.asarray(w, np.float32)
    b = np.asarray(b, np.float32)
    wqkT = w[0 : 2 * D].T.astype(ml_dtypes.bfloat16)  # [D, 256]
    wvT = w[2 * D : 3 * D].T.astype(ml_dtypes.bfloat16)  # [D, 128]
    bqk = np.ascontiguousarray(np.stack([b[0:D], b[D : 2 * D]], axis=1))  # [D, 2]
    bv = b[2 * D : 3 * D]  # [D], folded into xT on the host
    return wqkT, wvT, bqk, bv


class _PjrtStage:
    """Cached sharded PJRT executable for one Bass program (8-core SPMD)."""

    def __init__(self, nc):
        import jax
        from jax.sharding import Mesh, PartitionSpec
        from jax.experimental.shard_map import shard_map
        from concourse import bass2jax, mybir as _mybir

        bass2jax.install_neuronx_cc_hook()
        self.nc = nc
        part_name = nc.partition_id_tensor.name if nc.partition_id_tensor else None
        in_names, out_names, out_avals = [], [], []
        for alloc in nc.m.functions[0].allocations:
            if not isinstance(alloc, _mybir.MemoryLocationSet):
                continue
            name = alloc.memorylocations[0].name
            if alloc.kind == "ExternalInput":
                if name != part_name:
                    in_names.append(name)
            elif alloc.kind == "ExternalOutput":
                out_names.append(name)
                out_avals.append(
                    jax.core.ShapedArray(
                        tuple(alloc.tensor_shape), _mybir.dt.np(alloc.dtype)
                    )
                )
        self.in_names, self.out_names, self.out_avals = in_names, out_names, out_avals
        n_params = len(in_names)
        all_names = list(in_names + out_names)
        if part_name is not None:
            all_names.append(part_name)
        all_names = tuple(all_names)

        def _body(*args):
            operands = list(args)
            if part_name is not None:
                operands.append(bass2jax.partition_id_tensor())
            return tuple(
                bass2jax._bass_exec_p.bind(
                    *operands,
                    out_avals=tuple(out_avals),
                    in_names=all_names,
                    out_names=tuple(out_names),
                    lowering_input_output_aliases=(),
                    sim_require_finite=True,
                    sim_require_nnan=True,
                    nc=nc,
                )
            )

        devices = jax.devices()[:NCORES]
        mesh = Mesh(np.asarray(devices), ("core",))
        nio = n_params + len(out_names)
        self._fn = jax.jit(
            shard_map(
                _body,
                mesh=mesh,
                in_specs=(PartitionSpec("core"),) * nio,
                out_specs=(PartitionSpec("core"),) * len(out_names),
                check_rep=False,
            ),
            donate_argnums=tuple(range(n_params, nio)),
            keep_unused=True,
        )

    def concat_inputs(self, in_maps):
        return [
            np.concatenate([np.asarray(m[name]) for m in in_maps], axis=0)
            for name in self.in_names
        ]

    def run(self, concat_in):
        zeros = [
            np.zeros((NCORES * a.shape[0], *a.shape[1:]), a.dtype)
            for a in self.out_avals
        ]
        out = self._fn(*concat_in, *zeros)
        return [o for o in out]

    def __call__(self, in_maps):
        out = self.run(self.concat_inputs(in_maps))
        a = self.out_avals[0]
        return np.asarray(out[0]).reshape(NCORES, *a.shape)


_stage_runners = {}


def _get_runner(R, Lr):
    key = (R, Lr)
    if key not in _stage_runners:
        _stage_runners[key] = _PjrtStage(_get_stage(R, Lr))
    return _stage_runners[key]


def _run_stage(R, Lr, shards_cm, wqkT, wvT, bqk, bv):
    """shards_cm: list of 8 channel-major [D, N] f32 arrays. Returns
    [8, N, D] f32 (upcast from device bf16)."""
    in_maps = []
    for xs in shards_cm:
        xTb = xs.T + bv[None, :]  # fold v bias into the residual input
        in_maps.append(
            {
                "x_bf": xs.astype(ml_dtypes.bfloat16),
                "xT": np.ascontiguousarray(xTb).astype(ml_dtypes.bfloat16),
                "wqkT": wqkT,
                "wvT": wvT,
                "bqk": bqk,
            }
        )
    return _get_runner(R, Lr)(in_maps).astype(np.float32)


def kernel(**inputs):
    x = np.asarray(inputs["x"], np.float32)  # [1, D, S, L]
    g1 = np.asarray(inputs["gamma1"], np.float32)
    b1 = np.asarray(inputs["beta1"], np.float32)
    g2 = np.asarray(inputs["gamma2"], np.float32)
    b2 = np.asarray(inputs["beta2"], np.float32)

    # ---- stage 1: row attention, shard over S ----
    wqkT, wvT, bqk, bv = _prep_weights(inputs["w_row"], inputs["b_row"])
    Rs = S // NCORES
    shards = [
        np.ascontiguousarray(x[0][:, c * Rs : (c + 1) * Rs, :]).reshape(D, Rs * L)
        for c in range(NCORES)
    ]
    xn1 = _run_stage(Rs, L, shards, wqkT, wvT, bqk, bv)  # [8, Rs*L, D]
    out1 = xn1.reshape(S, L, D) * g1[None, None, :] + b1[None, None, :]

    # ---- stage 2: col attention, shard over L, per-core layout [D, l, s] ----
    wqkT, wvT, bqk, bv = _prep_weights(inputs["w_col"], inputs["b_col"])
    Rl = L // NCORES
    shards = [
        np.ascontiguousarray(
            out1[:, c * Rl : (c + 1) * Rl, :].transpose(2, 1, 0)
        ).reshape(D, Rl * S)
        for c in range(NCORES)
    ]
    xn2 = _run_stage(Rl, S, shards, wqkT, wvT, bqk, bv)  # [8, Rl*S, D]
    full = np.concatenate(
        [xn2[c].reshape(Rl, S, D) for c in range(NCORES)], axis=0
    )  # [L, S, D]
    out = full.transpose(1, 0, 2) * g2[None, None, :] + b2[None, None, :]  # [S, L, D]
    return np.ascontiguousarray(out.transpose(2, 0, 1))[None].astype(np.float32)


# revision 58
# speedup vs baseline: 1.0991x; 1.0105x over previous
"""AxialSelfAttention2d Trainium2 kernel (v4 = v1 skeleton + surgical wins).

Strategy (8 NeuronCores, SPMD, two launches):
  - Stage 1 (row attention, attend along L): shard over S -> 16 rows/core.
  - Host reshard, apply gamma1/beta1 exactly.
  - Stage 2 (col attention, attend along S): shard over L -> 32 cols/core,
    per-core layout [D, l, s] so both stages run the same kernel
    parameterized by (R rows, Lr row-length).

Device kernel per stage (per core, N = R*Lr = 4096 positions):
  inputs : x_bf [D, N] bf16 channel-major, xT [N, D] bf16 position-major
           with the v-bias pre-added (softmax weights sum to 1, so b_v
           joins the residual exactly), wqkT [D, 256] bf16, wvT [D, 128]
           bf16 (no bias, no ones), bqk [D, 2] f32.
  output : out [N, D] bf16 = LayerNorm(x + attn(x)) without gamma/beta.

  vs the original baseline:
  - single activation-table load at t=0 (Exp/Ln/Identity all masked into
    natural_log_exp_and_others + a dependency-free warmup exp).
  - a tunable fraction of the per-group exps runs on DVE via the
    Schraudolph bit-trick (i16 = s*A + B viewed as bf16; the constant
    factor cancels in softmax), balancing ACT vs DVE.
  - v eviction is a strided pure copy on ACT (b_v folded into xT on the
    host; denominator ones columns memset once), freeing DVE time.
  - y/y2/xn/xT are bf16: the residual add runs in DVE 2x mode and the
    xT-in/out DMAs halve.
"""

import os

os.environ.setdefault("MYCRO_LOCAL_CACHE", "1")

import numpy as np
import ml_dtypes

import concourse.bass as bass
import concourse.bacc as bacc
import concourse.tile as tile
from concourse import mybir

H, CH, D = 4, 32, 128
S, L = 128, 256
EPS = 1e-5
NCORES = 8
F32, BF16, I16 = mybir.dt.float32, mybir.dt.bfloat16, mybir.dt.int16
AF = mybir.ActivationFunctionType
OP = mybir.AluOpType

TRACE = False
SCH_A = 128.0 / float(np.log(2.0))
SCH_B = 16250.5
# groups whose exp runs on DVE (Schraudolph), per stage: {Lr: set of g}
SCH_GROUPS = {256: set(), 128: set()}
RES_ENGINE = "vector"  # residual add engine: vector | gpsimd
XN_ENGINE = {256: "gpsimd", 128: "gpsimd"}  # LN scale engine per stage
V_EVICT = {256: "vector", 128: "vector"}  # v eviction engine per stage
FIN_BLOCKS = 4  # finalize/store granularity (blocks)
K_EVICT_ACT = True  # stage2: k eviction on ACT (vs DVE)
EXP_BUFS = 4
X_CHUNKS = 4
DIV_PRIO = 60
EXP_PRIO = 60

_nc_cache = {}


def _stage_body(tc, d, R, Lr):
    nc = tc.nc
    N = R * Lr
    JB = Lr // 128          # j-blocks per attention row (2 for Lr=256, 1 for 128)
    NB = N // 128           # 32 position blocks
    RPT = 512 // (Lr * JB)  # attention rows per score tile (1 / 4)
    M = RPT * JB            # score subunits per bank == y-blocks per group
    G = R // RPT            # number of score groups
    sch_groups = SCH_GROUPS[Lr]

    import contextlib

    with contextlib.ExitStack() as ctx:
        cpool = ctx.enter_context(tc.tile_pool(name="consts", bufs=1))
        big = ctx.enter_context(tc.tile_pool(name="big", bufs=1))
        sm = ctx.enter_context(tc.tile_pool(name="small", bufs=1))
        pp = ctx.enter_context(tc.tile_pool(name="ps", bufs=1, space="PSUM"))

        # ---- warmup: hoist the single act-table load to t=0 ----
        wu = cpool.tile([128, 1], F32)
        nc.vector.memset(wu[:], 0.0)
        wu2 = cpool.tile([128, 1], F32)
        nc.scalar.activation(wu2[:], wu[:], AF.Exp)

        # ---- constants (wqk first: the first matmul needs it + x chunk 0) ----
        wqk_sb = cpool.tile([128, 256], BF16)
        nc.sync.dma_start(out=wqk_sb[:], in_=d["wqkT"][:])
        wv_sb = cpool.tile([128, 128], BF16)
        bqk_sb = cpool.tile([128, 2], F32)
        eps_sb = cpool.tile([128, 1], F32)
        nc.vector.memset(eps_sb[:], EPS)

        # ---- inputs ----
        x_sb = big.tile([128, N], BF16)
        XC = N // X_CHUNKS
        nc.sync.dma_start(out=x_sb[:, 0:XC], in_=d["x_bf"][:, 0:XC])
        nc.sync.dma_start(out=wv_sb[:], in_=d["wvT"][:])
        nc.sync.dma_start(out=bqk_sb[:], in_=d["bqk"][:])
        for q in range(1, X_CHUNKS):
            nc.sync.dma_start(
                out=x_sb[:, q * XC : (q + 1) * XC],
                in_=d["x_bf"][:, q * XC : (q + 1) * XC],
            )
        xT_sb = big.tile([128, NB, 128], BF16)
        xT_v = d["xT"].rearrange("(nb p) d -> p nb d", p=128)

        def emit_xT(q):  # deferred: residual input isn't needed until late
            nc.sync.dma_start(
                out=xT_sb[:, q * 8 : (q + 1) * 8, :], in_=xT_v[:, q * 8 : (q + 1) * 8, :]
            )

        # ---- persistent intermediates ----
        qk_sb = big.tile([128, 2, N], BF16)
        v_sb = big.tile([128, NB, 4, 33], BF16)
        # denominator ones columns, written once (evictions fill [.., 0:32])
        nc.vector.memset(v_sb[:, :, :, 32:33], 1.0)
        y_sb = big.tile([128, NB, 128], BF16)
        y2_sb = big.tile([128, NB, 128], BF16)
        xn_sb = big.tile([128, NB, 128], BF16)
        mv_sb = sm.tile([128, NB, 2], F32)
        rstd_sb = sm.tile([128, NB], F32)

        def emit_qk(cc):
            for ob in range(2):
                mmps = pp.tile([128, 4, 512], F32, tag="ps", bufs=2, name=f"qkps{ob}{cc}")
                for b in range(4):
                    nc.tensor.matmul(
                        mmps[:, b, :],
                        lhsT=wqk_sb[:, ob * 128 : (ob + 1) * 128],
                        rhs=x_sb[:, (cc * 4 + b) * 512 : (cc * 4 + b + 1) * 512],
                        start=True,
                        stop=True,
                    )
                outv = qk_sb[:, ob, cc * 2048 : (cc + 1) * 2048].rearrange(
                    "p (a b) -> p a b", b=512
                )
                on_act = (ob == 0) or (K_EVICT_ACT and JB == 1)
                if on_act:
                    nc.scalar.activation(
                        outv, mmps[:], AF.Identity, bias=bqk_sb[:, ob : ob + 1]
                    )
                else:
                    nc.vector.tensor_scalar(
                        outv, mmps[:], bqk_sb[:, ob : ob + 1], 0.0, OP.add, OP.add
                    )

        def emit_qk_head():
            # small first chunk (512 cols) so group 0's scores start ~4us
            # earlier; the rest of chunk 0 follows in a second tile.
            for ob in range(2):
                ps = pp.tile([128, 4, 512], F32, tag="ps", bufs=2, name=f"qkh{ob}")
                nc.tensor.matmul(
                    ps[:, 0, :],
                    lhsT=wqk_sb[:, ob * 128 : (ob + 1) * 128],
                    rhs=x_sb[:, 0:512],
                    start=True,
                    stop=True,
                )
                outv = qk_sb[:, ob, 0:512]
                if ob == 0:
                    nc.scalar.activation(
                        outv, ps[:, 0, :], AF.Identity, bias=bqk_sb[:, 0:1]
                    )
                else:
                    nc.vector.tensor_scalar(
                        outv, ps[:, 0, :], bqk_sb[:, 1:2], 0.0, OP.add, OP.add
                    )
            for ob in range(2):
                ps = pp.tile([128, 4, 512], F32, tag="ps", bufs=2, name=f"qkr{ob}")
                for b in range(1, 4):
                    nc.tensor.matmul(
                        ps[:, b, :],
                        lhsT=wqk_sb[:, ob * 128 : (ob + 1) * 128],
                        rhs=x_sb[:, b * 512 : (b + 1) * 512],
                        start=True,
                        stop=True,
                    )
                outv = qk_sb[:, ob, 512:2048].rearrange("p (a b) -> p a b", b=512)
                if ob == 0:
                    nc.scalar.activation(
                        outv, ps[:, 1:4, :], AF.Identity, bias=bqk_sb[:, 0:1]
                    )
                else:
                    nc.vector.tensor_scalar(
                        outv, ps[:, 1:4, :], bqk_sb[:, 1:2], 0.0, OP.add, OP.add
                    )

        def emit_v_head():
            # first 2 position blocks (group 0's keys), then the rest of
            # chunk 0, so AV(0) isn't gated by a full 8-block eviction.
            ps = pp.tile([128, 4, 512], F32, tag="ps", bufs=2, name="vh")
            vv = ps.rearrange("p b (s x) -> p (b s) x", x=256)
            for u in range(2):
                nc.tensor.matmul(
                    vv[:, u, 0:128],
                    lhsT=x_sb[:, u * 128 : (u + 1) * 128],
                    rhs=wv_sb[:],
                    start=True,
                    stop=True,
                )
            inv = vv[:, 0:2, 0:128].rearrange("p u (h c) -> p u h c", c=32)
            nc.vector.tensor_copy(v_sb[:, 0:2, :, 0:32], inv)
            ps2 = pp.tile([128, 4, 512], F32, tag="ps", bufs=2, name="vr")
            vv2 = ps2.rearrange("p b (s x) -> p (b s) x", x=256)
            for u in range(2, 8):
                nc.tensor.matmul(
                    vv2[:, u, 0:128],
                    lhsT=x_sb[:, u * 128 : (u + 1) * 128],
                    rhs=wv_sb[:],
                    start=True,
                    stop=True,
                )
            inv2 = vv2[:, 2:8, 0:128].rearrange("p u (h c) -> p u h c", c=32)
            nc.vector.tensor_copy(v_sb[:, 2:8, :, 0:32], inv2)

        def emit_v(t):
            vps = pp.tile([128, 4, 512], F32, tag="ps", bufs=2, name=f"vps{t}")
            vv = vps.rearrange("p b (s x) -> p (b s) x", x=256)
            for u in range(8):
                nb = t * 8 + u
                nc.tensor.matmul(
                    vv[:, u, 0:128],
                    lhsT=x_sb[:, nb * 128 : (nb + 1) * 128],
                    rhs=wv_sb[:],
                    start=True,
                    stop=True,
                )
            inv = vv[:, :, 0:128].rearrange("p u (h c) -> p u h c", c=32)
            outv = v_sb[:, t * 8 : (t + 1) * 8, :, 0:32]
            if V_EVICT[Lr] == "scalar":
                nc.scalar.activation(outv, inv, AF.Identity)
            else:
                nc.vector.tensor_copy(outv, inv)

        # ---- attention + LN ----
        sc_tiles = {}
        ex_tiles = {}

        def emit_scores_exp(g):
            rows = [g * RPT + rp for rp in range(RPT)]
            sc = pp.tile([128, 4, M, Lr], F32, tag="ps", bufs=2, name=f"sc{g}")
            sc_tiles[g] = sc
            for rp, r in enumerate(rows):
                for jb in range(JB):
                    kk = rp * JB + jb
                    for h in range(4):
                        nc.tensor.matmul(
                            sc[:, h, kk, :],
                            lhsT=qk_sb[
                                32 * h : 32 * h + 32,
                                1,
                                r * Lr + jb * 128 : r * Lr + (jb + 1) * 128,
                            ],
                            rhs=qk_sb[32 * h : 32 * h + 32, 0, r * Lr : (r + 1) * Lr],
                            start=True,
                            stop=True,
                            tile_position=(32 * h, 0),
                        )
            ex_i = sm.tile([128, 4, M, Lr], I16, tag="exp", bufs=EXP_BUFS, name=f"ex{g}")
            ex = ex_i[:].bitcast(BF16)
            ex_tiles[g] = ex
            with tc.high_priority(offset=EXP_PRIO) if EXP_PRIO else contextlib.nullcontext():
                if g in sch_groups:
                    nc.vector.tensor_scalar(
                        ex_i[:], sc[:], SCH_A, SCH_B, OP.mult, OP.add
                    )
                else:
                    nc.scalar.activation(ex, sc[:], AF.Exp)

        def emit_post(g):
            rows = [g * RPT + rp for rp in range(RPT)]
            sc = sc_tiles[g]
            ex = ex_tiles[g]
            sc_flat = sc.rearrange("p h m i -> p h (m i)")
            # AV: unit u -> y block nb = g*M + u; av psum reuses sc bank u.
            for u in range(M):
                r = rows[u // JB]
                ib = u % JB
                for h in range(4):
                    for jb in range(JB):
                        kk = (u // JB) * JB + jb
                        nc.tensor.matmul(
                            sc_flat[:, u, 33 * h : 33 * h + 33],
                            lhsT=ex[:, h, kk, ib * 128 : (ib + 1) * 128],
                            rhs=v_sb[:, r * JB + jb, h, :],
                            start=(jb == 0),
                            stop=(jb == JB - 1),
                        )
            # divide: y = num * (1/denom), batched over all M units.
            av4 = sc_flat[:, 0:M, 0:132].rearrange("p u (h c) -> p u h c", c=33)
            rt = sm.tile([128, M, 4], F32, tag="rt", bufs=4, name=f"rt{g}")
            yv = y_sb[:, g * M : (g + 1) * M, :].rearrange("p u (h c) -> p u h c", c=32)
            with tc.high_priority(offset=DIV_PRIO):
                nc.vector.reciprocal(rt[:], av4[:, :, :, 32])
                nc.vector.tensor_tensor(
                    yv, av4[:, :, :, 0:32],
                    rt[:, :, :, None].to_broadcast([128, M, 4, 32]), OP.mult,
                )
            # residual add: all-bf16 SBUF -> DVE 2x mode
            getattr(nc, RES_ENGINE).tensor_tensor(
                y2_sb[:, g * M : (g + 1) * M, :],
                y_sb[:, g * M : (g + 1) * M, :],
                xT_sb[:, g * M : (g + 1) * M, :],
                OP.add,
            )
            # LN stats per block
            for u in range(M):
                nb = g * M + u
                st = sm.tile([128, 6], F32, tag="st", bufs=6, name=f"st{g}_{u}")
                nc.vector.bn_stats(st[:], y2_sb[:, nb, :])
                nc.vector.bn_aggr(mv_sb[:, nb, :], st[:])

            # finalize + store every FIN blocks
            FIN = FIN_BLOCKS
            done = (g + 1) * M
            if done % FIN == 0:
                gg = done // FIN - 1
                sl = slice(gg * FIN, gg * FIN + FIN)
                # rstd = exp(-0.5*ln(var+eps)): Ln and Exp share one table set
                lnv = sm.tile([128, FIN], F32, tag="std", bufs=2, name=f"lnv{gg}")
                nc.scalar.activation(lnv[:], mv_sb[:, sl, 1], AF.Ln, bias=eps_sb[:])
                nc.scalar.activation(rstd_sb[:, sl], lnv[:], AF.Exp, scale=-0.5)
                xn_eng = "vector" if done == NB else XN_ENGINE[Lr]  # tail on DVE
                for nb in range(gg * FIN, gg * FIN + FIN):
                    getattr(nc, xn_eng).tensor_scalar(
                        xn_sb[:, nb, :],
                        y2_sb[:, nb, :],
                        mv_sb[:, nb, 0:1],
                        rstd_sb[:, nb : nb + 1],
                        OP.subtract,
                        OP.mult,
                    )
                out_v = d["out"].rearrange("(nb p) d -> p nb d", p=128)
                nc.sync.dma_start(out=out_v[:, sl, :], in_=xn_sb[:, sl, :])

        # driver: v1 order -- AV(g) directly follows scores(g) (separated by
        # the exp wait, which lets the PE quadrant streams drain before the
        # full-array AV matmul; reordering these crashes the exec unit).
        emit_qk_head()
        emit_v(0)
        emit_v(1)
        for q in range(4):
            emit_xT(q)
        for half in range(2):
            for g in range(half * (G // 2), (half + 1) * (G // 2)):
                emit_scores_exp(g)
                emit_post(g)
            if half == 0:
                emit_qk(1)
                emit_v(2)
                emit_v(3)


def _build_stage(R, Lr):
    N = R * Lr
    nc = bacc.Bacc("TRN2", target_bir_lowering=False, debug=False)
    d = {
        "x_bf": nc.dram_tensor("x_bf", [D, N], BF16, kind="ExternalInput").ap(),
        "xT": nc.dram_tensor("xT", [N, D], BF16, kind="ExternalInput").ap(),
        "wqkT": nc.dram_tensor("wqkT", [D, 2 * D], BF16, kind="ExternalInput").ap(),
        "wvT": nc.dram_tensor("wvT", [D, D], BF16, kind="ExternalInput").ap(),
        "bqk": nc.dram_tensor("bqk", [D, 2], F32, kind="ExternalInput").ap(),
        "out": nc.dram_tensor("out", [N, D], BF16, kind="ExternalOutput").ap(),
    }
    with tile.TileContext(nc) as tc:
        _stage_body(tc, d, R, Lr)
    _compile_with_shared_act_table(nc)
    return nc


def _compile_with_shared_act_table(nc):
    """Steer the act-table-load pass to the one set containing Exp, Ln AND
    Identity (natural_log_exp_and_others) so exactly one table load runs."""
    import concourse.hw_specs as hws

    orig = hws.get_activation_tables
    orig_bacc = bacc.get_activation_tables
    tabs = dict(orig(nc.m.arch))
    want = {AF.Exp, AF.Ln}
    mask = {AF.Exp, AF.Ln, AF.Identity}
    shared = [n for n, fs in tabs.items() if want <= fs]
    if shared:
        keep = shared[0]
        masked = {n: (fs if n == keep else (fs - mask)) for n, fs in tabs.items()}
        patched = lambda arch, _m=masked: _m
        hws.get_activation_tables = patched
        bacc.get_activation_tables = patched
    try:
        nc.compile()
    finally:
        hws.get_activation_tables = orig
        bacc.get_activation_tables = orig_bacc


def _get_stage(R, Lr):
    key = (R, Lr)
    if key not in _nc_cache:
        _nc_cache[key] = _build_stage(R, Lr)
    return _nc_cache[key]


def _prep_weights(w, b):
    """Host-side packing of the [384, 128] qkv conv weights."""
    w = np.asarray(w, np.float32)
    b = np.asarray(b, np.float32)
    wqkT = w[0 : 2 * D].T.astype(ml_dtypes.bfloat16)  # [D, 256]
    wvT = w[2 * D : 3 * D].T.astype(ml_dtypes.bfloat16)  # [D, 128]
    bqk = np.ascontiguousarray(np.stack([b[0:D], b[D : 2 * D]], axis=1))  # [D, 2]
    bv = b[2 * D : 3 * D]  # [D], folded into xT on the host
    return wqkT, wvT, bqk, bv


class _PjrtStage:
    """Cached sharded PJRT executable for one Bass program (8-core SPMD)."""

    def __init__(self, nc):
        import jax
        from jax.sharding import Mesh, PartitionSpec
        from jax.experimental.shard_map import shard_map
        from concourse import bass2jax, mybir as _mybir

        bass2jax.install_neuronx_cc_hook()
        self.nc = nc
        part_name = nc.partition_id_tensor.name if nc.partition_id_tensor else None
        in_names, out_names, out_avals = [], [], []
        for alloc in nc.m.functions[0].allocations:
            if not isinstance(alloc, _mybir.MemoryLocationSet):
                continue
            name = alloc.memorylocations[0].name
            if alloc.kind == "ExternalInput":
                if name != part_name:
                    in_names.append(name)
            elif alloc.kind == "ExternalOutput":
                out_names.append(name)
                out_avals.append(
                    jax.core.ShapedArray(
                        tuple(alloc.tensor_shape), _mybir.dt.np(alloc.dtype)
                    )
                )
        self.in_names, self.out_names, self.out_avals = in_names, out_names, out_avals
        n_params = len(in_names)
        all_names = list(in_names + out_names)
        if part_name is not None:
            all_names.append(part_name)
        all_names = tuple(all_names)

        def _body(*args):
            operands = list(args)
            if part_name is not None:
                operands.append(bass2jax.partition_id_tensor())
            return tuple(
                bass2jax._bass_exec_p.bind(
                    *operands,
                    out_avals=tuple(out_avals),
                    in_names=all_names,
                    out_names=tuple(out_names),
                    lowering_input_output_aliases=(),
                    sim_require_finite=True,
                    sim_require_nnan=True,
                    nc=nc,
                )
            )

        devices = jax.devices()[:NCORES]
        mesh = Mesh(np.asarray(devices), ("core",))
        nio = n_params + len(out_names)
        self._fn = jax.jit(
            shard_map(
                _body,
                mesh=mesh,
                in_specs=(PartitionSpec("core"),) * nio,
                out_specs=(PartitionSpec("core"),) * len(out_names),
                check_rep=False,
            ),
            donate_argnums=tuple(range(n_params, nio)),
            keep_unused=True,
        )

    def concat_inputs(self, in_maps):
        return [
            np.concatenate([np.asarray(m[name]) for m in in_maps], axis=0)
            for name in self.in_names
        ]

    def run(self, concat_in):
        zeros = [
            np.zeros((NCORES * a.shape[0], *a.shape[1:]), a.dtype)
            for a in self.out_avals
        ]
        out = self._fn(*concat_in, *zeros)
        return [o for o in out]

    def __call__(self, in_maps):
        out = self.run(self.concat_inputs(in_maps))
        a = self.out_avals[0]
        return np.asarray(out[0]).reshape(NCORES, *a.shape)


_stage_runners = {}


def _get_runner(R, Lr):
    key = (R, Lr)
    if key not in _stage_runners:
        _stage_runners[key] = _PjrtStage(_get_stage(R, Lr))
    return _stage_runners[key]


def _run_stage(R, Lr, shards_cm, wqkT, wvT, bqk, bv):
    """shards_cm: list of 8 channel-major [D, N] f32 arrays. Returns
    [8, N, D] f32 (upcast from device bf16)."""
    in_maps = []
    for xs in shards_cm:
        xTb = xs.T + bv[None, :]  # fold v bias into the residual input
        in_maps.append(
            {
                "x_bf": xs.astype(ml_dtypes.bfloat16),
                "xT": np.ascontiguousarray(xTb).astype(ml_dtypes.bfloat16),
                "wqkT": wqkT,
                "wvT": wvT,
                "bqk": bqk,
            }
        )
    return _get_runner(R, Lr)(in_maps).astype(np.float32)


def kernel(**inputs):
    x = np.asarray(inputs["x"], np.float32)  # [1, D, S, L]
    g1 = np.asarray(inputs["gamma1"], np.float32)
    b1 = np.asarray(inputs["beta1"], np.float32)
    g2 = np.asarray(inputs["gamma2"], np.float32)
    b2 = np.asarray(inputs["beta2"], np.float32)

    # ---- stage 1: row attention, shard over S ----
    wqkT, wvT, bqk, bv = _prep_weights(inputs["w_row"], inputs["b_row"])
    Rs = S // NCORES
    shards = [
        np.ascontiguousarray(x[0][:, c * Rs : (c + 1) * Rs, :]).reshape(D, Rs * L)
        for c in range(NCORES)
    ]
    xn1 = _run_stage(Rs, L, shards, wqkT, wvT, bqk, bv)  # [8, Rs*L, D]
    out1 = xn1.reshape(S, L, D) * g1[None, None, :] + b1[None, None, :]

    # ---- stage 2: col attention, shard over L, per-core layout [D, l, s] ----
    wqkT, wvT, bqk, bv = _prep_weights(inputs["w_col"], inputs["b_col"])
    Rl = L // NCORES
    shards = [
        np.ascontiguousarray(
            out1[:, c * Rl : (c + 1) * Rl, :].transpose(2, 1, 0)
        ).reshape(D, Rl * S)
        for c in range(NCORES)
    ]
    xn2 = _run_stage(Rl, S, shards, wqkT, wvT, bqk, bv)  # [8, Rl*S, D]
    full = np.concatenate(
        [xn2[c].reshape(Rl, S, D) for c in range(NCORES)], axis=0
    )  # [L, S, D]
    out = full.transpose(1, 0, 2) * g2[None, None, :] + b2[None, None, :]  # [S, L, D]
    return np.ascontiguousarray(out.transpose(2, 0, 1))[None].astype(np.float32)
